# revision 29
# baseline (speedup 1.0000x reference)
"""Trainium2 Bass kernel for nn_CosineSimilarityLayer.

out = l2norm_rows(x) @ l2norm_rows_over_N(W)       x:[4096,512]  W:[512,5994]

Math:  out[b,n] = xscale[b] * sum_d x[b,d] * wscale[d] * W[d,n]
  xscale[b] = rsqrt(max(sum_d x[b,d]^2, eps))   (folded into PSUM eviction)
  wscale[d] = rsqrt(max(sum_n W[d,n]^2, eps))   (folded into transposed x)

Sharding: data-parallel over batch - 8 cores x [512, 512] x-shards, W
replicated.  No collectives: a measured 2KB AllReduce costs ~40us here.

wscale gates every matmul, so the W-norm scan is split three ways to
finish ~18us, each consumer fed by its own slice of a 3MB fp8 shadow in
DMA arrival order:
  * DVE squares+reduces 910 columns (2-pass, from a [D,n] fp8 slice),
  * ACT fused-Square+accums 1500 columns (same slice, lands second),
  * the PE computes the remaining 3584 columns as a Gram diagonal:
    DoubleRow fp8 matmuls accumulate diag blocks of W8T^T @ W8T over 14
    row-tile pairs, chasing the shadow DMA; a DVE identity-mask+reduce
    extracts the diagonals.  PSUM start=True zeroing is bank-granular,
    so the four 512B gram regions share one pre-memset bank and
    accumulate with start=False.
fp8 norm error is ~1e-3 relative on wscale -> ~1e-4 on out (gate 2e-2).

x arrives both as x (for ACT's row-sumsq) and pre-transposed xT (host
layout transform) - no on-device transposes.  All IO bf16; host casts,
out is upcast on the host.  Matmul: group-outer / bt-inner so W chunks
are consumed ~3x slower than they land; first groups are 1 and 2 chunks
so the PE starts as soon as chunk 0 lands; dt-outer inside a group
reuses the stationary across PSUM banks.  Eviction (scale by xscale,
round to bf16) alternates ACT/DVE; out DMA on the scalar HWDGE ring.
"""

import os
import sys
import types
from contextlib import ExitStack

import numpy as np


def _ensure_axon_hooks():
    """bass_utils' trace path imports antenv.axon_hooks, which some images
    lack.  Provide it (wired to the ctypes NTFF hook when available) so
    BASS_TRACE=1 profiles instead of crashing.  No-op when already present."""
    try:
        import antenv.axon_hooks  # noqa: F401
        return
    except ImportError:
        pass
    try:
        import antenv
    except ImportError:
        return
    m = types.ModuleType("antenv.axon_hooks")
    holder = {"h": None}
    m.set_axon_ntff_profile_hook = lambda h: holder.__setitem__("h", h)
    m.get_axon_ntff_profile_hook = lambda: holder["h"]
    sys.modules["antenv.axon_hooks"] = m
    antenv.axon_hooks = m
    try:
        from trn_agent_boot.trn_boot import _ntff_profile_via_ctypes
        so = "/opt/axon/libaxon_pjrt.so"
        if os.path.exists(so):
            m.set_axon_ntff_profile_hook(_ntff_profile_via_ctypes(so))
    except Exception:
        pass


_ensure_axon_hooks()

import ml_dtypes
import concourse.bass as bass
import concourse.tile as tile
from concourse import bacc, mybir
from concourse.bass_utils import run_bass_kernel_spmd
from concourse.masks import make_identity

F32 = mybir.dt.float32
BF16 = mybir.dt.bfloat16
FP8 = mybir.dt.float8e4
AF = mybir.ActivationFunctionType

B, D, N = 4096, 512, 5994
NCORES = 8
P = 128
BSH = B // NCORES          # 512 rows of x per core
BT = BSH // P              # 4 b-tiles
DT = D // P                # 4 d-tiles (contraction)
CHUNK = 512                # output n-chunk (one PSUM bank of fp32)
EPS = 1e-12

# ---- W-norm hybrid split ----
NPAIR = 14                 # gram row-tile pairs: 14*256 = 3584 rows
NGRAM = NPAIR * 2 * P      # 3584 = W columns covered by the PE gram
NACT = 1500                # ACT's column span (fused Square+accum)
NDVE = N - NGRAM - NACT    # 910, DVE 2-pass
NENG = NACT + NDVE         # engine-shadow columns (2410)

CHUNKS = []
_n0 = 0
while _n0 < N:
    CHUNKS.append((_n0, min(CHUNK, N - _n0)))
    _n0 += CHUNK
NCH = len(CHUNKS)          # 12
# small leading groups (PE starts on chunk 0 alone) and a small final
# group (short tail); 3-chunk groups in between
_GIDX = [[0], [1, 2], [3, 4, 5], [6, 7, 8], [9, 10], [11]]
GROUPS = []                # (start, width, chunk indices)
for _ix in _GIDX:
    _c = [CHUNKS[i] for i in _ix]
    GROUPS.append((_c[0][0], _c[-1][0] + _c[-1][1] - _c[0][0], _ix))


def _build():
    nc = bacc.Bacc("TRN2", target_bir_lowering=False, debug=False,
                   num_devices=NCORES)

    x_d = nc.dram_tensor("x", [BSH, D], BF16, kind="ExternalInput").ap()
    xt_d = nc.dram_tensor("xT", [D, BSH], BF16, kind="ExternalInput").ap()
    w16_d = nc.dram_tensor("W16", [D, N], BF16, kind="ExternalInput").ap()
    w8t_d = nc.dram_tensor("W8T", [P, NPAIR, 2, D], FP8,
                           kind="ExternalInput").ap()
    w8e_d = nc.dram_tensor("W8E", [D, NENG], FP8, kind="ExternalInput").ap()
    o_d = nc.dram_tensor("out", [BSH, N], BF16, kind="ExternalOutput").ap()
    sdbg_d = nc.dram_tensor("SDBG", [P, DT], F32, kind="ExternalOutput").ap()

    x_r = x_d.rearrange("(t p) d -> p t d", p=P)        # [128, 4, 512]
    xt_r = xt_d.rearrange("(t p) b -> p t b", p=P)      # [128, 4, 512]
    w16_r = w16_d.rearrange("(t p) n -> p t n", p=P)    # [128, 4, 5994]
    w8e_r = w8e_d.rearrange("(t p) n -> p t n", p=P)    # [128, 4, 2410]
    o_r = o_d.rearrange("(t p) n -> p t n", p=P)        # [128, 4, 5994]

    with tile.TileContext(nc) as tc, ExitStack() as ctx:
        const = ctx.enter_context(tc.tile_pool(name="const", bufs=1))
        xp = ctx.enter_context(tc.tile_pool(name="xp", bufs=1))
        sq = ctx.enter_context(tc.tile_pool(name="sq", bufs=2))
        sc = ctx.enter_context(tc.tile_pool(name="sc", bufs=1))
        xt = ctx.enter_context(tc.tile_pool(name="xt", bufs=1))
        wp = ctx.enter_context(tc.tile_pool(name="wp", bufs=1))
        ostp = ctx.enter_context(tc.tile_pool(name="ostp", bufs=4))
        gp = ctx.enter_context(tc.tile_pool(name="gp", bufs=1, space="PSUM"))
        mm = ctx.enter_context(tc.tile_pool(name="mm", bufs=6, space="PSUM"))

        # ---- input DMAs, issued up front in stream order ----
        w8e = wp.tile([P, DT, NENG], FP8)
        for t in range(DT):
            nc.sync.dma_start(w8e[:, t, NACT:], w8e_r[:, t, NACT:])
        for t in range(DT):
            nc.sync.dma_start(w8e[:, t, :NACT], w8e_r[:, t, :NACT])
        w8t = wp.tile([P, NPAIR, 2, D], FP8)
        for j0, j1 in ((0, 5), (5, 10), (10, NPAIR)):
            nc.sync.dma_start(w8t[:, j0:j1], w8t_d[:, j0:j1])
        x_sb = xp.tile([P, BT, D], BF16)
        nc.sync.dma_start(x_sb, x_r)
        xtf = xt.tile([P, DT, BSH], BF16, tag="xtf")
        for t in range(DT):
            nc.sync.dma_start(xtf[:, t, :], xt_r[:, t, :])
        w16 = wp.tile([P, DT, N], BF16)
        for g0, gw, _ in GROUPS:
            for t in range(DT):
                nc.sync.dma_start(w16[:, t, g0:g0 + gw],
                                  w16_r[:, t, g0:g0 + gw])

        # ---- preload both ACT tables before any data lands ----
        dum = sc.tile([P, 2], F32)
        dum2 = sc.tile([P, 2], F32)
        nc.scalar.activation(dum[:, 0:1], dum[:, 1:2], AF.Square)
        nc.scalar.activation(dum2[:, 0:1], dum[:, 0:1], AF.Sqrt)
        identity = const.tile([P, P], BF16)
        make_identity(nc, identity)

        # ---- W norm partials ----
        # slot 0: PE gram diag, slot 1: ACT, slot 2: DVE
        wsqp = sc.tile([P, DT, 3], F32)

        # DVE 2-pass over its engine-shadow slice (lands first)
        for t in range(DT):
            trd = sq.tile([P, NDVE], BF16, tag="trd")
            nc.vector.tensor_tensor(trd, w8e[:, t, NACT:], w8e[:, t, NACT:],
                                    mybir.AluOpType.mult)
            nc.vector.reduce_sum(wsqp[:, t, 2:3], trd,
                                 axis=mybir.AxisListType.X)

        # ACT fused Square+accum over its slice
        for t in range(DT):
            tra = sq.tile([P, NACT], BF16, tag="tra")
            nc.scalar.activation(tra, w8e[:, t, :NACT], AF.Square,
                                 accum_out=wsqp[:, t, 1:2])

        # PE gram: first in the PE stream (paces the wscale chain).
        # PSUM start=True zeroing is bank-granular (2KB), so the four
        # 512B accumulation regions share one pre-memset bank and all
        # matmuls use start=False.
        gps = gp.tile([P, DT, P], F32)
        nc.vector.memset(gps, 0.0)
        for j in range(NPAIR):
            for db in range(DT):
                blk = w8t[:, j, :, db * P:(db + 1) * P]
                nc.tensor.matmul(gps[:, db, :], blk, blk,
                                 perf_mode=mybir.MatmulPerfMode.DoubleRow,
                                 start=False, stop=(j == NPAIR - 1),
                                 skip_group_check=True)
        for db in range(DT):
            dg = sq.tile([P, P], F32, tag="diag", name=f"dg{db}")
            nc.vector.tensor_tensor(dg, gps[:, db, :], identity,
                                    mybir.AluOpType.mult)
            nc.vector.reduce_sum(wsqp[:, db, 0:1], dg,
                                 axis=mybir.AxisListType.X)

        # ---- wscale = rsqrt(max(sum(partials), eps)) ----
        wsq = sc.tile([P, DT, 1], F32)
        nc.vector.reduce_sum(wsq, wsqp, axis=mybir.AxisListType.X)
        nc.gpsimd.dma_start(sdbg_d, wsq[:, :, 0])
        wmx = sc.tile([P, DT, 1], F32)
        nc.vector.tensor_scalar_max(wmx, wsq, EPS)
        wsr = sc.tile([P, DT, 1], F32)
        nc.scalar.sqrt(wsr, wmx)
        wsc = sc.tile([P, DT, 1], F32)
        nc.vector.reciprocal(wsc, wsr)

        # ---- fold wscale into the host-transposed x ----
        xtr = xt.tile([P, DT, BSH], BF16, tag="xtr")
        for dt in range(DT):
            nc.vector.tensor_scalar_mul(xtr[:, dt, :], xtf[:, dt, :],
                                        wsc[:, dt, :])

        # ---- xscale = rsqrt(max(rowsumsq(x), eps)) on ACT (fused) ----
        xsq = sc.tile([P, BT], F32)
        for bt in range(BT):
            trx = sq.tile([P, D], BF16, tag="trx")
            nc.scalar.activation(trx, x_sb[:, bt, :], AF.Square,
                                 accum_out=xsq[:, bt:bt + 1])
        xmx = sc.tile([P, BT], F32)
        nc.vector.tensor_scalar_max(xmx, xsq, EPS)
        xsr = sc.tile([P, BT], F32)
        nc.scalar.sqrt(xsr, xmx)
        xsc = sc.tile([P, BT], F32)
        nc.vector.reciprocal(xsc, xsr)

        # ---- matmul: group outer (W arrival order), bt inner; dt outer
        # within a group so the stationary is reused across banks ----
        evict = [0]
        for g, (g0, gw, gix) in enumerate(GROUPS):
            grp = [CHUNKS[i] for i in gix]
            for bt in range(BT):
                pss = [mm.tile([P, CHUNK], F32, tag="ps", name=f"ps{c}")
                       for c in range(len(grp))]
                for dt in range(DT):
                    for c, (n0, nw) in enumerate(grp):
                        nc.tensor.matmul(
                            pss[c][:, :nw],
                            xtr[:, dt, bt * P:(bt + 1) * P],
                            w16[:, dt, n0:n0 + nw],
                            start=(dt == 0), stop=(dt == DT - 1))
                ost = ostp.tile([P, 3 * CHUNK], BF16, tag="ost")
                for c, (n0, nw) in enumerate(grp):
                    # GPSIMD cannot read PSUM: alternate ACT/DVE.
                    dst = ost[:, n0 - g0:n0 - g0 + nw]
                    if evict[0] % 2 == 0:
                        nc.scalar.activation(dst, pss[c][:, :nw], AF.Copy,
                                             scale=xsc[:, bt:bt + 1])
                    else:
                        nc.vector.tensor_scalar_mul(dst, pss[c][:, :nw],
                                                    xsc[:, bt:bt + 1])
                    evict[0] += 1
                # scalar = ACT HWDGE ring; gpsimd DMA is the slow SW queue
                nc.scalar.dma_start(o_r[:, bt, g0:g0 + gw], ost[:, :gw])

    nc.compile()
    return nc


LAST_RESULT = None


def kernel(x: np.ndarray, W: np.ndarray) -> np.ndarray:
    global LAST_RESULT
    x = np.ascontiguousarray(x, dtype=np.float32)
    W = np.ascontiguousarray(W, dtype=np.float32)
    assert x.shape == (B, D) and W.shape == (D, N)

    x16 = x.astype(ml_dtypes.bfloat16)
    W16 = np.ascontiguousarray(W.astype(ml_dtypes.bfloat16))
    W8E = np.ascontiguousarray(W[:, NGRAM:].astype(ml_dtypes.float8_e4m3))

    # gram shadow: W^T rows [0:NGRAM] interleaved to [128, pair, 2, D]:
    # partition p of pair j holds rows 256j+p and 256j+128+p.
    w8t = W.T[:NGRAM].astype(ml_dtypes.float8_e4m3)
    w8t = np.ascontiguousarray(
        w8t.reshape(NPAIR, 2, P, D).transpose(2, 0, 1, 3))

    nc = _build()

    in_maps = []
    for c in range(NCORES):
        xs = x16[c * BSH:(c + 1) * BSH]
        in_maps.append({"x": np.ascontiguousarray(xs),
                        "xT": np.ascontiguousarray(xs.T),
                        "W16": W16, "W8T": w8t, "W8E": W8E})

    res = run_bass_kernel_spmd(nc, in_maps, core_ids=list(range(NCORES)))
    LAST_RESULT = res
    out = np.concatenate([res.results[c]["out"] for c in range(NCORES)],
                         axis=0)
    return out.astype(np.float32)
